# revision 4
# baseline (speedup 1.0000x reference)
"""Fused top-k/top-p/sampling kernel for Trainium2 (8 NeuronCores).

Contract: kernel(**inputs) takes FULL inputs (logits [256,128000] f32,
top_ks [256] int, top_ps [256] f32, q [256,128000] f32) and returns the
FULL output tuple (selected_idx [256] int32, selected_logits [256,128000]
f32), matching reference semantics.

Strategy (rows sharded 32/core across 8 cores, pure data parallel):
  Phase 1 (device): per row, exact top-256 (values + indices) via the
      gpsimd `topk` ucode instruction. Only ~16KB/core comes back.
  Host glue (O(rows*64)): stable top-64 order, replicate the reference's
      f32 top-k/top-p prefix decision on the 64-wide window, derive the
      per-row value cutoff v_cut and the sampled index (argmax of
      p/(q+eps) over the <=63 kept positions, gathering q on host).
  Phase 2 (device): stream logits again and write
      out = x + 2*((x < v_cut) * -1.8e38)
      which is exactly x where x >= v_cut and exactly -inf elsewhere
      (the *2 overflows -3.6e38 to -inf in f32; 0*2+x == x is exact).
      Tie rows (elements == v_cut beyond the kept prefix) are patched on
      host using the exact candidate info from phase 1.

The decision math only needs the top-63 elements per row (top_ks < 64),
so q is never streamed on device and the device does two 16MB/core reads
plus one 16MB/core write: ~393MB of HBM traffic total across 8 cores.
"""

import os
import sys

if "/opt/trn_rl_repo" not in sys.path:
    sys.path.insert(0, "/opt/trn_rl_repo")

import numpy as np

R, V = 256, 128000
NCORES = 8
RPC = R // NCORES          # 32 rows per core
TOKS = 8                   # tokens per topk instruction
CALLS = RPC // TOKS        # 4 topk calls per core
KTOP = 256                 # candidates per row from device
TOPQ = 64                  # candidates actually used per row (top_ks < 64)
F2 = 2000                  # phase-2 free-dim tile size (per partition)
NCH = V // 4 // F2         # 16 chunks (4 partitions per row)
BIG = -1.8e38              # (x<t)*BIG*2 + x  ->  -inf exactly when dropped
EPS = 1e-08
NEG_INF = np.float32(-np.inf)

_cache = {}
last_exec_ns = {}          # phase -> exec_time_ns (filled when tracing)


def _build_phase1():
    import concourse.bass as bass
    import concourse.mybir as mybir
    from concourse import library_config

    nc = bass.Bass()
    x = nc.dram_tensor("x", [RPC, V], mybir.dt.float32, kind="ExternalInput")
    cand = nc.dram_tensor(
        "cand", [CALLS, 128, 32], mybir.dt.uint32, kind="ExternalOutput"
    )
    with (
        nc.semaphore("dma_a") as dma_a,
        nc.semaphore("dma_b") as dma_b,
        nc.semaphore("dma_c") as dma_c,
        nc.semaphore("comp") as comp,
        nc.sbuf_tensor("xb0", [128, V // 16], mybir.dt.float32) as xb0,
        nc.sbuf_tensor("xb1", [128, V // 16], mybir.dt.float32) as xb1,
        nc.sbuf_tensor("cb0", [128, 32], mybir.dt.uint32) as cb0,
        nc.sbuf_tensor("cb1", [128, 32], mybir.dt.uint32) as cb1,
        nc.sbuf_tensor("cb2", [128, 32], mybir.dt.uint32) as cb2,
        nc.sbuf_tensor("cb3", [128, 32], mybir.dt.uint32) as cb3,
    ):
        xbufs = [xb0, xb1]
        cbufs = [cb0, cb1, cb2, cb3]
        in_sems = [dma_a, dma_b]

        with nc.Block() as block:

            @block.sync
            def _(sync):
                for t in range(CALLS):
                    if t >= 2:
                        # buffer t%2 is free once topk t-2 completed
                        sync.wait_ge(comp, t - 1)
                    sync.dma_start(
                        xbufs[t % 2][:, :],
                        x[t * TOKS : (t + 1) * TOKS, :].rearrange(
                            "r (p c) -> (r p) c", p=16
                        ),
                    ).then_inc(in_sems[t % 2], 16)
                for t in range(CALLS):
                    sync.wait_ge(comp, t + 1)
                    sync.dma_start(cand[t, :, :], cbufs[t][:, :]).then_inc(dma_c, 16)
                sync.wait_ge(dma_a, 32)
                sync.wait_ge(dma_b, 32)
                sync.wait_ge(dma_c, 64)

            @block.gpsimd
            def _(g):
                g.load_library(library_config.topk)
                for t in range(CALLS):
                    g.wait_ge(in_sems[t % 2], 16 * (t // 2 + 1))
                    g.topk(
                        cbufs[t][:, :],
                        xbufs[t % 2][:, :],
                        tokens=TOKS,
                        vocab_size=V,
                        k=KTOP,
                    ).then_inc(comp, 1)

    return nc


def _build_phase2():
    import concourse.bass as bass
    import concourse.mybir as mybir
    from concourse.tile import TileContext

    f32 = mybir.dt.float32
    alu = mybir.AluOpType

    nc = bass.Bass()
    x = nc.dram_tensor("x", [RPC, V], f32, kind="ExternalInput")
    vcut = nc.dram_tensor("vcut", [128, 1], f32, kind="ExternalInput")
    y = nc.dram_tensor("y", [RPC, V], f32, kind="ExternalOutput")

    xr = x[:, :].rearrange("r (p n) -> (r p) n", p=4)
    yr = y[:, :].rearrange("r (p n) -> (r p) n", p=4)

    with TileContext(nc) as tc:
        with (
            tc.tile_pool(name="xp", bufs=3) as xp,
            tc.tile_pool(name="mp", bufs=3) as mp,
            tc.tile_pool(name="vp", bufs=1) as vp,
        ):
            vt = vp.tile([128, 1], f32)
            nc.sync.dma_start(vt[:, :], vcut[:, :])
            for c in range(NCH):
                xt = xp.tile([128, F2], f32, tag="x")
                nc.sync.dma_start(xt[:, :], xr[:, c * F2 : (c + 1) * F2])
                mt = mp.tile([128, F2], f32, tag="m")
                # m = (x < vcut) * BIG   (2x DVE mode: single tensor source)
                nc.vector.tensor_scalar(
                    mt[:, :], xt[:, :], vt[:, :], BIG, alu.is_lt, alu.mult
                )
                # x = (m * 2) + x   -> x where kept, -inf where dropped
                nc.vector.scalar_tensor_tensor(
                    xt[:, :], mt[:, :], 2.0, xt[:, :], alu.mult, alu.add
                )
                nc.sync.dma_start(yr[:, c * F2 : (c + 1) * F2], xt[:, :])
    return nc


def _get(name):
    if name not in _cache:
        _cache[name] = _build_phase1() if name == "p1" else _build_phase2()
    return _cache[name]


def _run_spmd(nc, in_maps, phase):
    """Run on the 8 NeuronCores; returns list of per-core output dicts."""
    if os.environ.get("BASS_KERNEL_SIM"):
        from concourse.bass_interp import CoreSim

        results = []
        for m in in_maps:
            sim = CoreSim(nc, require_finite=False)
            for k, v in m.items():
                sim.tensor(k)[:] = v
            sim.simulate()
            out = {}
            for alloc in nc.m.functions[0].allocations:
                try:
                    kind = alloc.kind
                except AttributeError:
                    continue
                if kind == "ExternalOutput":
                    name = alloc.memorylocations[0].name
                    out[name] = np.array(sim.tensor(name))
            last_exec_ns[phase] = int(sim.time)
            results.append(out)
        return results

    from concourse.bass_utils import run_bass_kernel_spmd

    trace = bool(os.environ.get("BASS_KERNEL_TRACE"))
    res = run_bass_kernel_spmd(
        nc,
        in_maps,
        core_ids=list(range(NCORES)),
        trace=trace,
    )
    if res.exec_time_ns is not None:
        last_exec_ns[phase] = int(res.exec_time_ns)
    return res.results


def _decode_candidates(cands):
    """cands: per-core [CALLS,128,32] uint32 -> vals [R,256] f32 asc, idx [R,256]."""
    vals = np.empty((R, KTOP), np.float32)
    idxs = np.empty((R, KTOP), np.int64)
    for c, cd in enumerate(cands):
        blk = cd.reshape(CALLS, TOKS, 16, 32)
        v = np.ascontiguousarray(blk[..., :16]).reshape(CALLS * TOKS, KTOP)
        i = np.ascontiguousarray(blk[..., 16:]).reshape(CALLS * TOKS, KTOP)
        vals[c * RPC : (c + 1) * RPC] = v.view(np.float32)
        idxs[c * RPC : (c + 1) * RPC] = i.astype(np.int64)
    return vals, idxs


def _jax_cpu():
    import jax

    return jax, jax.devices("cpu")[0]


def _window_decision(svals, kvec, top_ps):
    """Replicate the reference's f32 softmax/cumsum/top-p decision on the
    64-wide sorted window (bitwise-faithful to the full-width computation:
    the nonzero prefix occupies the same dyadic subtrees). Returns
    (n_keep [R], mask [R,TOPQ], boundary_margin [R])."""
    jax, cpu = _jax_cpu()
    import jax.numpy as jnp

    with jax.default_device(cpu):
        sv = jnp.asarray(svals)
        kk = jnp.asarray(kvec.astype(np.int32))
        tp = jnp.asarray(top_ps)
        ranks = jnp.arange(TOPQ, dtype=jnp.int32)
        mask_k = ranks[None, :] < kk[:, None]
        tl = jnp.where(mask_k, sv, -jnp.inf)
        probs = jax.nn.softmax(tl, axis=-1)
        cum = jnp.cumsum(probs, axis=-1)
        prev = cum - probs
        mask_p = prev <= tp[:, None]
        mask = mask_k & mask_p
        mask = mask.at[:, 0].set(True)
        n_keep = mask.sum(axis=-1).astype(jnp.int32)
        margin = jnp.min(
            jnp.where(mask_k, jnp.abs(prev - tp[:, None]), jnp.inf), axis=-1
        )
        return (
            np.asarray(n_keep),
            np.asarray(mask),
            np.asarray(margin),
        )


def _reference_rows(rows, logits, top_ks, top_ps, q):
    """Literal reference math (jnp f32, CPU) for a small set of rows.
    Returns (sel_idx [n] int32, sel_logits [n, V] f32)."""
    jax, cpu = _jax_cpu()
    import jax.numpy as jnp

    with jax.default_device(cpu):
        lg = jnp.asarray(logits[rows])
        k = jnp.asarray(top_ks[rows].astype(np.int32))
        tp = jnp.asarray(top_ps[rows])
        qq = jnp.asarray(q[rows])
        order = jnp.argsort(-lg, axis=-1)
        sorted_logits = jnp.take_along_axis(lg, order, axis=-1)
        k = jnp.where(k <= 0, V, k)
        ranks = jnp.arange(V, dtype=jnp.int32)
        mask_k = ranks[None, :] < k[:, None]
        topk_logits = jnp.where(mask_k, sorted_logits, -jnp.inf)
        probs = jax.nn.softmax(topk_logits, axis=-1)
        cum = jnp.cumsum(probs, axis=-1)
        mask_p = (cum - probs) <= tp[:, None]
        mask = mask_k & mask_p
        mask = mask.at[:, 0].set(True)
        masked_sorted = jnp.where(mask, sorted_logits, -jnp.inf)
        inv = jnp.argsort(order, axis=-1)
        selected_logits = jnp.take_along_axis(masked_sorted, inv, axis=-1)
        final_probs = jax.nn.softmax(selected_logits, axis=-1)
        sel = jnp.argmax(final_probs / (qq + EPS), axis=-1).astype(jnp.int32)
        return np.asarray(sel), np.asarray(selected_logits)


def _reference_sample_rows(rows, sel_logits, q):
    """Exact reference sampling (argmax of softmax(sel_logits)/(q+eps))
    for specific rows, given the final selected_logits."""
    jax, cpu = _jax_cpu()
    import jax.numpy as jnp

    with jax.default_device(cpu):
        sl = jnp.asarray(sel_logits[rows])
        qq = jnp.asarray(q[rows])
        fp = jax.nn.softmax(sl, axis=-1)
        return np.asarray(jnp.argmax(fp / (qq + EPS), axis=-1).astype(jnp.int32))


def kernel(logits, top_ks, top_ps, q):
    logits = np.ascontiguousarray(np.asarray(logits, dtype=np.float32))
    kvec = np.asarray(top_ks).astype(np.int64).reshape(R)
    top_ps = np.asarray(top_ps, dtype=np.float32).reshape(R)
    q = np.asarray(q, dtype=np.float32)

    # ---- phase 1: exact top-256 per row on device ----
    nc1 = _get("p1")
    in1 = [{"x": logits[c * RPC : (c + 1) * RPC]} for c in range(NCORES)]
    r1 = _run_spmd(nc1, in1, "p1")
    vals_asc, idxs_asc = _decode_candidates([r["cand"] for r in r1])

    # top-64 in reference order: value desc, index asc (stable ties)
    ordw = np.lexsort((idxs_asc, -vals_asc), axis=-1)[:, :TOPQ]
    rows_i = np.arange(R)[:, None]
    svals = np.take_along_axis(vals_asc, ordw, 1)
    sidx = np.take_along_axis(idxs_asc, ordw, 1)

    bad = np.zeros(R, dtype=bool)
    # device-value sanity: values must equal logits at the reported indices
    bad |= np.any(logits[rows_i, sidx] != svals, axis=1)
    # duplicate indices within a row's top-64 (ucode tie pathology)
    ss = np.sort(sidx, axis=1)
    bad |= np.any(ss[:, 1:] == ss[:, :-1], axis=1)
    # k outside the top-64 window
    bad |= (kvec <= 0) | (kvec >= TOPQ)

    # ---- host decision: n_keep / v_cut per row (f32, reference-faithful) ----
    kk = np.where(bad, 1, kvec).astype(np.int64)
    n_keep, mask, margin = _window_decision(svals, kk, top_ps)
    # non-prefix mask would break the threshold construction
    prefix = np.arange(TOPQ)[None, :] < n_keep[:, None]
    bad |= np.any(mask != prefix, axis=1)
    bad |= margin < 1e-5
    n_keep = np.clip(n_keep, 1, TOPQ)
    v_cut = np.take_along_axis(svals, n_keep[:, None] - 1, 1)[:, 0]

    # ---- phase 2: threshold-materialize the output on device ----
    nc2 = _get("p2")
    in2 = []
    for c in range(NCORES):
        vc = np.repeat(v_cut[c * RPC : (c + 1) * RPC], 4).reshape(128, 1)
        vc = np.ascontiguousarray(vc.astype(np.float32))
        in2.append({"x": logits[c * RPC : (c + 1) * RPC], "vcut": vc})
    r2 = _run_spmd(nc2, in2, "p2")
    out = np.concatenate([r["y"] for r in r2], axis=0)

    # tie fix: elements equal to v_cut beyond the kept prefix -> -inf
    beyond = np.arange(TOPQ)[None, :] >= n_keep[:, None]
    tiefix = beyond & (svals == v_cut[:, None])
    for r_ in np.nonzero(np.any(tiefix, axis=1))[0]:
        if not bad[r_]:
            out[r_, sidx[r_, tiefix[r_]]] = NEG_INF

    # ---- sampling: argmax p/(q+eps) over kept positions (f64 + guard) ----
    kept = ~beyond
    sv64 = svals.astype(np.float64)
    e = np.exp(sv64 - sv64[:, :1]) * kept
    p = e / e.sum(axis=1, keepdims=True)
    qg = q[rows_i, sidx]
    den = (qg + np.float32(EPS)).astype(np.float64)
    ratio = np.where(kept, p / den, -1.0)
    best = ratio.max(axis=1)
    # winner = lowest vocab index among exact-max ties
    is_best = ratio == best[:, None]
    sel_idx = np.where(is_best, sidx, np.int64(V + 1)).min(axis=1).astype(np.int32)
    # near-tie guard: second-distinct ratio too close -> exact fallback
    second = np.where(is_best, -np.inf, ratio).max(axis=1)
    with np.errstate(invalid="ignore", divide="ignore"):
        close = (best - second) <= 1e-5 * np.abs(best)
    n_best = is_best.sum(axis=1)
    sample_rows = np.nonzero((close | (n_best > 1)) & ~bad)[0]
    if sample_rows.size:
        sel_idx[sample_rows] = _reference_sample_rows(sample_rows, out, q)

    # ---- full fallback for anomalous rows (expected: none) ----
    bad_rows = np.nonzero(bad)[0]
    if bad_rows.size:
        fb_idx, fb_logits = _reference_rows(bad_rows, logits, kvec, top_ps, q)
        sel_idx[bad_rows] = fb_idx
        out[bad_rows] = fb_logits

    return sel_idx.astype(np.int32), out.astype(np.float32, copy=False)


# revision 16
# speedup vs baseline: 1.0203x; 1.0203x over previous
"""Fused top-k/top-p/sampling kernel for Trainium2 (8 NeuronCores).

Contract: kernel(**inputs) takes FULL inputs (logits [256,128000] f32,
top_ks [256] int, top_ps [256] f32, q [256,128000] f32) and returns the
FULL output tuple (selected_idx [256] int32, selected_logits [256,128000]
f32), matching reference semantics.

Strategy (rows sharded 32/core across 8 cores, pure data parallel):
  Phase 1 (device): per row, exact top-256 (values + indices) via the
      gpsimd `topk` ucode instruction. Only ~16KB/core comes back.
  Host glue (O(rows*64)): stable top-64 order, replicate the reference's
      f32 top-k/top-p prefix decision on the 64-wide window, derive the
      per-row value cutoff v_cut and the sampled index (argmax of
      p/(q+eps) over the <=63 kept positions, gathering q on host).
  Phase 2 (device): stream logits again and write
      out = x + 2*((x < v_cut) * -1.8e38)
      which is exactly x where x >= v_cut and exactly -inf elsewhere
      (the *2 overflows -3.6e38 to -inf in f32; 0*2+x == x is exact).
      Tie rows (elements == v_cut beyond the kept prefix) are patched on
      host using the exact candidate info from phase 1.

The decision math only needs the top-63 elements per row (top_ks < 64),
so q is never streamed on device and the device does two 16MB/core reads
plus one 16MB/core write: ~393MB of HBM traffic total across 8 cores.
"""

import os
import sys

if "/opt/trn_rl_repo" not in sys.path:
    sys.path.insert(0, "/opt/trn_rl_repo")

import numpy as np

R, V = 256, 128000
NCORES = 8
RPC = R // NCORES          # 32 rows per core
HALVES = 2                 # topk ISA encodes n as u16 -> split rows in half
VH = V // HALVES           # 64000 per half-row "token"
VROWS = RPC * HALVES       # 64 virtual tokens per core
TOKS = 8                   # tokens per topk instruction
CALLS = VROWS // TOKS      # 8 topk calls per core
KTOP = 256                 # candidates per half-row from device
TOPQ = 64                  # candidates actually used per row (top_ks < 64)
F2 = 2000                  # phase-2 free-dim tile size (per partition)
NCH = V // 4 // F2         # 16 chunks (4 partitions per row)
BIG = -1.8e38              # (x<t)*BIG*2 + x  ->  -inf exactly when dropped
EPS = 1e-08
NEG_INF = np.float32(-np.inf)

_cache = {}
last_exec_ns = {}          # phase -> exec_time_ns (filled when tracing)


def _build_phase1():
    from contextlib import ExitStack

    import concourse.mybir as mybir
    from concourse import bacc

    nc = bacc.Bacc()
    x = nc.dram_tensor("x", [RPC, V], mybir.dt.float32, kind="ExternalInput")
    cand = nc.dram_tensor(
        "cand", [CALLS, 128, 32], mybir.dt.uint32, kind="ExternalOutput"
    )
    with (
        nc.semaphore("dma_a") as dma_a,
        nc.semaphore("dma_b") as dma_b,
        nc.semaphore("dma_c") as dma_c,
        nc.semaphore("comp") as comp,
        nc.sbuf_tensor("xb0", [128, VH // 16], mybir.dt.float32) as xb0,
        nc.sbuf_tensor("xb1", [128, VH // 16], mybir.dt.float32) as xb1,
        ExitStack() as stack,
    ):
        cb = [
            stack.enter_context(
                nc.sbuf_tensor(f"cb{t}", [128, 32], mybir.dt.uint32)
            )
            for t in range(CALLS)
        ]
        xbufs = [xb0, xb1]
        in_sems = [dma_a, dma_b]
        # rows x halves as 64 virtual tokens of width VH (contiguous)
        xv = x[:, :].rearrange("r (h n) -> (r h) n", h=HALVES)

        with nc.Block() as block:

            @block.sync
            def _(sync):
                for t in range(CALLS):
                    if t >= 2:
                        # buffer t%2 is free once topk t-2 completed
                        sync.wait_ge(comp, t - 1)
                    sync.dma_start(
                        xbufs[t % 2][:, :],
                        xv[t * TOKS : (t + 1) * TOKS, :].rearrange(
                            "v (p c) -> (v p) c", p=16
                        ),
                    ).then_inc(in_sems[t % 2], 16)
                for t in range(CALLS):
                    sync.wait_ge(comp, t + 1)
                    sync.dma_start(cand[t, :, :], cb[t][:, :]).then_inc(dma_c, 16)
                sync.wait_ge(dma_a, 16 * (CALLS // 2))
                sync.wait_ge(dma_b, 16 * (CALLS // 2))
                sync.wait_ge(dma_c, 16 * CALLS)

            @block.gpsimd
            def _(g):
                for t in range(CALLS):
                    g.wait_ge(in_sems[t % 2], 16 * (t // 2 + 1))
                    g.topk(
                        cb[t][:, :],
                        xbufs[t % 2][:, :],
                        tokens=TOKS,
                        vocab_size=VH,
                        k=KTOP,
                    ).then_inc(comp, 1)

    nc.finalize()
    return nc


def _build_phase2():
    import concourse.mybir as mybir
    from concourse import bacc
    from concourse.tile import TileContext

    f32 = mybir.dt.float32
    alu = mybir.AluOpType

    nc = bacc.Bacc()
    x = nc.dram_tensor("x", [RPC, V], f32, kind="ExternalInput")
    vcut = nc.dram_tensor("vcut", [128, 1], f32, kind="ExternalInput")
    y = nc.dram_tensor("y", [RPC, V], f32, kind="ExternalOutput")

    xr = x[:, :].rearrange("r (p n) -> (r p) n", p=4)
    yr = y[:, :].rearrange("r (p n) -> (r p) n", p=4)

    with TileContext(nc) as tc:
        with (
            tc.tile_pool(name="xp", bufs=3) as xp,
            tc.tile_pool(name="mp", bufs=3) as mp,
            tc.tile_pool(name="vp", bufs=1) as vp,
        ):
            vt = vp.tile([128, 1], f32)
            nc.sync.dma_start(vt[:, :], vcut[:, :])
            for c in range(NCH):
                xt = xp.tile([128, F2], f32, tag="x")
                nc.sync.dma_start(xt[:, :], xr[:, c * F2 : (c + 1) * F2])
                mt = mp.tile([128, F2], f32, tag="m")
                # m = (x < vcut) * BIG   (2x DVE mode: single tensor source)
                nc.vector.tensor_scalar(
                    mt[:, :], xt[:, :], vt[:, :], BIG, alu.is_lt, alu.mult
                )
                # x = (m * 2) + x   -> x where kept, -inf where dropped
                nc.vector.scalar_tensor_tensor(
                    xt[:, :], mt[:, :], 2.0, xt[:, :], alu.mult, alu.add
                )
                nc.sync.dma_start(yr[:, c * F2 : (c + 1) * F2], xt[:, :])
    nc.finalize()
    return nc


def _get(name):
    if name not in _cache:
        _cache[name] = _build_phase1() if name == "p1" else _build_phase2()
    return _cache[name]


def _run_spmd(nc, in_maps, phase):
    """Run on the 8 NeuronCores; returns list of per-core output dicts."""
    if os.environ.get("BASS_KERNEL_SIM"):
        from concourse.bass_interp import CoreSim

        results = []
        for m in in_maps:
            sim = CoreSim(nc, require_finite=False)
            for k, v in m.items():
                sim.tensor(k)[:] = v
            sim.simulate()
            out = {}
            for alloc in nc.m.functions[0].allocations:
                try:
                    kind = alloc.kind
                except AttributeError:
                    continue
                if kind == "ExternalOutput":
                    name = alloc.memorylocations[0].name
                    out[name] = np.array(sim.tensor(name))
            last_exec_ns[phase] = int(sim.time)
            results.append(out)
        return results

    from concourse.bass_utils import run_bass_kernel_spmd

    trace = bool(os.environ.get("BASS_KERNEL_TRACE"))
    res = run_bass_kernel_spmd(
        nc,
        in_maps,
        core_ids=list(range(NCORES)),
        trace=trace,
    )
    if res.exec_time_ns is not None:
        last_exec_ns[phase] = int(res.exec_time_ns)
    return res.results


def _decode_candidates(cands):
    """cands: per-core [CALLS,128,32] uint32 (one top-256 per half-row)
    -> per full row: vals [R, 2*KTOP] f32, idx [R, 2*KTOP] int64 (global)."""
    vals = np.empty((R, HALVES * KTOP), np.float32)
    idxs = np.empty((R, HALVES * KTOP), np.int64)
    half_off = np.tile(
        np.repeat(np.arange(HALVES) * VH, KTOP)[None, :], (RPC, 1)
    )
    for c, cd in enumerate(cands):
        blk = cd.reshape(CALLS, TOKS, 16, 32)
        v = np.ascontiguousarray(blk[..., :16]).reshape(VROWS, KTOP)
        i = np.ascontiguousarray(blk[..., 16:]).reshape(VROWS, KTOP)
        # virtual row v = 2*r + h  ->  row-major [RPC, HALVES*KTOP]
        vals[c * RPC : (c + 1) * RPC] = v.view(np.float32).reshape(
            RPC, HALVES * KTOP
        )
        idxs[c * RPC : (c + 1) * RPC] = (
            i.astype(np.int64).reshape(RPC, HALVES * KTOP) + half_off
        )
    return vals, idxs


def _jax_cpu():
    import jax

    return jax, jax.devices("cpu")[0]


def _window_decision(svals, kvec, top_ps):
    """Replicate the reference's f32 softmax/cumsum/top-p decision on the
    64-wide sorted window (bitwise-faithful to the full-width computation:
    the nonzero prefix occupies the same dyadic subtrees). Returns
    (n_keep [R], mask [R,TOPQ], boundary_margin [R])."""
    jax, cpu = _jax_cpu()
    import jax.numpy as jnp

    with jax.default_device(cpu):
        sv = jnp.asarray(svals)
        kk = jnp.asarray(kvec.astype(np.int32))
        tp = jnp.asarray(top_ps)
        ranks = jnp.arange(TOPQ, dtype=jnp.int32)
        mask_k = ranks[None, :] < kk[:, None]
        tl = jnp.where(mask_k, sv, -jnp.inf)
        probs = jax.nn.softmax(tl, axis=-1)
        cum = jnp.cumsum(probs, axis=-1)
        prev = cum - probs
        mask_p = prev <= tp[:, None]
        mask = mask_k & mask_p
        mask = mask.at[:, 0].set(True)
        n_keep = mask.sum(axis=-1).astype(jnp.int32)
        margin = jnp.min(
            jnp.where(mask_k, jnp.abs(prev - tp[:, None]), jnp.inf), axis=-1
        )
        return (
            np.asarray(n_keep),
            np.asarray(mask),
            np.asarray(margin),
        )


def _reference_rows(rows, logits, top_ks, top_ps, q):
    """Literal reference math (jnp f32, CPU) for a small set of rows.
    Returns (sel_idx [n] int32, sel_logits [n, V] f32)."""
    jax, cpu = _jax_cpu()
    import jax.numpy as jnp

    with jax.default_device(cpu):
        lg = jnp.asarray(logits[rows])
        k = jnp.asarray(top_ks[rows].astype(np.int32))
        tp = jnp.asarray(top_ps[rows])
        qq = jnp.asarray(q[rows])
        order = jnp.argsort(-lg, axis=-1)
        sorted_logits = jnp.take_along_axis(lg, order, axis=-1)
        k = jnp.where(k <= 0, V, k)
        ranks = jnp.arange(V, dtype=jnp.int32)
        mask_k = ranks[None, :] < k[:, None]
        topk_logits = jnp.where(mask_k, sorted_logits, -jnp.inf)
        probs = jax.nn.softmax(topk_logits, axis=-1)
        cum = jnp.cumsum(probs, axis=-1)
        mask_p = (cum - probs) <= tp[:, None]
        mask = mask_k & mask_p
        mask = mask.at[:, 0].set(True)
        masked_sorted = jnp.where(mask, sorted_logits, -jnp.inf)
        inv = jnp.argsort(order, axis=-1)
        selected_logits = jnp.take_along_axis(masked_sorted, inv, axis=-1)
        final_probs = jax.nn.softmax(selected_logits, axis=-1)
        sel = jnp.argmax(final_probs / (qq + EPS), axis=-1).astype(jnp.int32)
        return np.asarray(sel), np.asarray(selected_logits)


def _reference_sample_rows(rows, sel_logits, q):
    """Exact reference sampling (argmax of softmax(sel_logits)/(q+eps))
    for specific rows, given the final selected_logits."""
    jax, cpu = _jax_cpu()
    import jax.numpy as jnp

    with jax.default_device(cpu):
        sl = jnp.asarray(sel_logits[rows])
        qq = jnp.asarray(q[rows])
        fp = jax.nn.softmax(sl, axis=-1)
        return np.asarray(jnp.argmax(fp / (qq + EPS), axis=-1).astype(jnp.int32))


def kernel(logits, top_ks, top_ps, q):
    logits = np.ascontiguousarray(np.asarray(logits, dtype=np.float32))
    kvec = np.asarray(top_ks).astype(np.int64).reshape(R)
    top_ps = np.asarray(top_ps, dtype=np.float32).reshape(R)
    q = np.asarray(q, dtype=np.float32)

    # ---- phase 1: exact top-256 per row on device ----
    nc1 = _get("p1")
    in1 = [{"x": logits[c * RPC : (c + 1) * RPC]} for c in range(NCORES)]
    r1 = _run_spmd(nc1, in1, "p1")
    vals_asc, idxs_asc = _decode_candidates([r["cand"] for r in r1])

    # top-64 in reference order: value desc, index asc (stable ties)
    ordw = np.lexsort((idxs_asc, -vals_asc), axis=-1)[:, :TOPQ]
    rows_i = np.arange(R)[:, None]
    svals = np.take_along_axis(vals_asc, ordw, 1)
    sidx = np.take_along_axis(idxs_asc, ordw, 1)

    bad = np.zeros(R, dtype=bool)
    # device-value sanity: values must equal logits at the reported indices
    bad |= np.any(logits[rows_i, sidx] != svals, axis=1)
    # duplicate indices within a row's top-64 (ucode tie pathology)
    ss = np.sort(sidx, axis=1)
    bad |= np.any(ss[:, 1:] == ss[:, :-1], axis=1)
    # k outside the top-64 window
    bad |= (kvec <= 0) | (kvec >= TOPQ)

    # ---- host decision: n_keep / v_cut per row (f32, reference-faithful) ----
    kk = np.where(bad, 1, kvec).astype(np.int64)
    n_keep, mask, margin = _window_decision(svals, kk, top_ps)
    # non-prefix mask would break the threshold construction
    prefix = np.arange(TOPQ)[None, :] < n_keep[:, None]
    bad |= np.any(mask != prefix, axis=1)
    bad |= margin < 1e-5
    n_keep = np.clip(n_keep, 1, TOPQ)
    v_cut = np.take_along_axis(svals, n_keep[:, None] - 1, 1)[:, 0]

    # ---- phase 2: threshold-materialize the output on device ----
    nc2 = _get("p2")
    in2 = []
    for c in range(NCORES):
        vc = np.repeat(v_cut[c * RPC : (c + 1) * RPC], 4).reshape(128, 1)
        vc = np.ascontiguousarray(vc.astype(np.float32))
        in2.append({"x": logits[c * RPC : (c + 1) * RPC], "vcut": vc})
    r2 = _run_spmd(nc2, in2, "p2")
    out = np.concatenate([r["y"] for r in r2], axis=0)

    # tie fix: elements equal to v_cut beyond the kept prefix -> -inf
    beyond = np.arange(TOPQ)[None, :] >= n_keep[:, None]
    tiefix = beyond & (svals == v_cut[:, None])
    for r_ in np.nonzero(np.any(tiefix, axis=1))[0]:
        if not bad[r_]:
            out[r_, sidx[r_, tiefix[r_]]] = NEG_INF

    # ---- sampling: argmax p/(q+eps) over kept positions (f64 + guard) ----
    kept = ~beyond
    sv64 = svals.astype(np.float64)
    e = np.exp(sv64 - sv64[:, :1]) * kept
    p = e / e.sum(axis=1, keepdims=True)
    qg = q[rows_i, sidx]
    den = (qg + np.float32(EPS)).astype(np.float64)
    ratio = np.where(kept, p / den, -1.0)
    best = ratio.max(axis=1)
    # winner = lowest vocab index among exact-max ties
    is_best = ratio == best[:, None]
    sel_idx = np.where(is_best, sidx, np.int64(V + 1)).min(axis=1).astype(np.int32)
    # near-tie guard: second-distinct ratio too close -> exact fallback
    second = np.where(is_best, -np.inf, ratio).max(axis=1)
    with np.errstate(invalid="ignore", divide="ignore"):
        close = (best - second) <= 1e-5 * np.abs(best)
    n_best = is_best.sum(axis=1)
    sample_rows = np.nonzero((close | (n_best > 1)) & ~bad)[0]
    if sample_rows.size:
        sel_idx[sample_rows] = _reference_sample_rows(sample_rows, out, q)

    # ---- full fallback for anomalous rows (expected: none) ----
    bad_rows = np.nonzero(bad)[0]
    if bad_rows.size:
        fb_idx, fb_logits = _reference_rows(bad_rows, logits, kvec, top_ps, q)
        sel_idx[bad_rows] = fb_idx
        out[bad_rows] = fb_logits

    return sel_idx.astype(np.int32), out.astype(np.float32, copy=False)
